# revision 1
# baseline (speedup 1.0000x reference)
"""Trainium2 Bass kernel for nn_LossFunction_103079215159 (triplet-style loss
with online hard-negative mining).

Math (B=8192 rows, D=256 features; x[:,0]=anchors x0, x[:,1]=positives x1):
  a = l2norm(x0), p = l2norm(x1)
  dist[i,j] = || a_i - p_j + eps ||  (via gemm expansion), diag masked +inf
  top5 smallest per row -> pick rank[i]-th (RNG-derived, data-independent)
  loss = mean relu(||a_i-p_i+eps||^2 - ||a_i-p_neg+eps||^2)

Reduction used here: with s[i,j] = <2*x0_i/||x0_i||, x1_j/||x1_j||> = 2*cos_ij,
  d2[i,j] = 2 - s[i,j] (+O(1e-6) eps terms that cancel / are negligible), so
  loss_i = relu(s_sel[i] - s_ii[i]) where s_sel is the rank[i]-th LARGEST
  masked s row value. sqrt never needed; per-row affine constants cancel.

Distribution: 8-way data parallel over anchor rows. Each core gets its
[1024, 256] anchor slab plus the positives matrix ROTATED by its row offset
(np.roll) so the self-match diagonal lands at identical local addresses on
every core (single SPMD program). Per core:
  - normalize both operand slabs on device (ACT: sum-of-squares, sqrt;
    DVE: reciprocal; GpSimd: row scaling)
  - PE-transpose the scaled operands to [D, *] layout (fp32r)
  - G = a'ance @ p~.T via fp32r matmuls into PSUM [128, 1024] granules
  - mask diag (DVE add of -3e38 identity), top-8 per row via DVE Max8
    directly from PSUM, hierarchical merge
  - one-hot select the rank-th value, subtract s_ii (row-dot on GpSimd),
    relu -> per-row losses -> DRAM
Host: input slicing/rotation, RNG one-hot (embedded constant), final mean.
"""

import base64

import numpy as np

B = 8192
D = 256
NCORES = 8
M = B // NCORES  # 1024 anchor rows per core
RB = M // 128  # 8 row blocks per core
NG = 8  # granules of 1024 cols each
GW = 1024  # granule width
CH = 16  # x1 chunks of 512 rows
CW = 512  # chunk rows

NEG_BIG = -3.0e38

# rank[i] in {0..4}: which of the 5 nearest negatives to use per row.
# Reproduces exactly (verified):
#   k1, k2 = jax.random.split(jax.random.key(1))
#   coin = jax.random.uniform(k1, (8192,)) < 0.5
#   rank = jnp.where(coin, 0, jax.random.randint(k2, (8192,), 0, 5))
_RANK_B64 = (
    "AAIEAAAAAAAAAAIAAwAAAAAAAAAAAAMAAAIAAAMABAAAAAAAAwACAAABAAQCBAADAAACAgAEAwAC"
    "AAMEAAAAAwEEAQMAAAIAAgAAAAAAAAAEAAQAAwAABAECAAIAAAAAAgADAAACAwQABAAAAgMAAgAE"
    "AwAAAgACAAECAAEAAAECAQEBAAAABAACBAAAAAAAAAEAAAAEAQAAAAIAAgADAAEAAAAAAQAAAQME"
    "AgAAAAEEAAAAAAMAAQAAAAAEAAAEAQAAAAAAAAAAAAAAAAADAQQAAAAAAgABAAAAAAADAAADAAQA"
    "AAAAAwMAAAAEAAAAAAAAAAEAAAMAAAAAAAQAAAACAgAEAQAAAAABAAADAgABAAIAAAAAAwQCAAAD"
    "AgAAAAADAgAAAQAABAAABAAAAAAAAAIAAAEABAADAAAAAAAEAAAAAQEBAAAAAAMAAAIAAAAAAAMA"
    "AwIDAAEAAQQAAAIAAAEEAAECAAIAAAEAAAADAAIAAQICAAABAgAAAQAAAAIAAAADAAEDBAAAAQEA"
    "AgAAAAAEBAAAAAEAAgECAAIEAAAABAAEAQIABAAAAAAAAAAAAAMBAQAAAAMCAgADAAIDAwQDBAAE"
    "AAAAAAAAAAEAAAEAAwMAAAAAAAAAAAABAAAAAAAAAAEAAAADAgMAAAMAAAAAAAMAAQAAAAAAAgAA"
    "BAAAAAMBAQABAAAAAAAAAAIAAwAAAgAEAwABAAAAAAAAAAAAAAIAAgABAgAEAAABAQIAAgIDAgAE"
    "AAAAAAAAAQAABAAEAAAAAAAAAQIAAgAAAAMAAQACAAAAAAADAAQAAQABBAAEAAMABAABAQADAQAA"
    "AgABAgAEAAIAAAAAAgAAAwAAAwAAAAAEAAAAAAEAAAAAAAIEAAAAAgAABAEAAgAAAAAAAAEAAAAC"
    "AAECBAADAAAAAQAAAAIAAAAAAgMAAAAAAQAAAAQAAAAAAAMEAwEAAgEAAAAAAAAABAADAQIDAAAA"
    "AAEAAwAAAgAAAAEAAgAAAAAAAgAAAAAABAAEAAACAAIAAAQAAgADAAEAAAQAAAACAAECAwIEAAAA"
    "BAQAAAQABAMAAAQAAwIAAQMAAAQAAAACAAAEAAAABAAAAAAAAAMBAAEAAAQDAAAAAAQDAAAAAAIA"
    "AAAEAwACAAQAAgACAAACAQQAAAQDAgQDAQAAAAAEAAADBAECBAAEAAEBAAAAAAEAAgAAAwAAAgAB"
    "AwAAAgAEBAAAAAIEAAAAAwACAAIBAAABAwQAAQAAAAQAAAAAAAIAAAEBAAIAAAAAAAEAAAAAAAEB"
    "AAAAAgACAAAAAAMAAwAAAAAABAMABAMAAQQBAAQCAAEDAAAAAAIAAAAEAAMDAAAEAAEAAQAAAAAA"
    "AAICBAABAQQEAAAAAAQAAQABAAEEAAACBAAAAAMAAAAABAAAAAEBAAICAAIAAAAAAAAEBAAAAAMC"
    "AAQDAAABAAQCAAEAAAAABAQEAAIBAAAAAgAEAAEAAAIEBAACAAIAAAAABAMDBAQAAAAAAAIAAgAA"
    "AAACAAABAwMDAAAAAAAAAAACAQAAAwAAAAAEAAAAAAMAAAAAAgMAAAICAAMAAAAEAAAAAAABAAAA"
    "AAABAAAAAAMAAAEEAAIDAAEBAAQAAAMCAAAAAAAEAAACAAMAAAACAwAAAwAEAAAAAAQAAwABAAAC"
    "AwAAAAEABAQBAAIAAAIAAwAEAAEAAAACAgAAAAEEAAQAAAADAAMDAAQDBAABBAACAwAAAAAEAAMA"
    "AgQABAIAAAAEAAQCAQMAAAIBAAIAAAQEAAACAAEAAAAAAAEAAAABAAEAAAAABAAAAAAABAADAAAA"
    "BAABBAABAAADAAAAAAAAAAAAAQAAAAAAAAMAAQAAAQACAAAAAAACAAMAAAMAAwIBAAAABAAAAAMA"
    "AAAAAAABAAABAQIBAAAAAgAAAAAEAAAAAAQAAAAAAwAAAAAAAgAAAAAAAAAAAAACAgAAAAABBAAA"
    "AwACAAEDAAAAAAQAAQACAAAEAAAAAgAAAAIAAAMBAAAAAAIEAwAAAAQAAAMAAAMAAAAAAAAAAAMC"
    "BAQAAAMAAAEBAQAAAAAAAAIAAAMAAAMAAAAAAAIABAAAAAABAgAAAAAEAAQCAAIAAAIDAAMBAAAA"
    "AwAAAQADAwABAAADAAAEAwAAAAAABAMAAAEAAAAAAAAAAAAAAAAAAAAAAAACAAAAAAICAgACAAMA"
    "AAACAwAAAAIAAQAAAAAEAQAAAgAEAAEAAwAEAAAAAAAAAAQAAwAAAwAAAAQEAgAAAAMEAAAAAAAB"
    "AwQAAgADAgEDAAQDAAAAAAIAAAAAAAAAAAAABAQAAAEEBAABAAAAAQQAAAAABAAAAAMCAAAAAAAD"
    "BAAAAAEEAwIAAAADAAAAAAAEAAIAAAMBAAADAAAAAAAAAgAAAAMCAAAEAgACAAADAAAAAwABBAAD"
    "AAIAAAAAAQAABAADAAAAAAQAAQABAAMAAwADAAAAAAAAAAMEAwADAwQBAAAAAAMAAAAAAAEDAAAE"
    "AQAAAAAAAgAAAQAAAAICAAIEAAABBAACAAABAgAAAQAABAIDAgAEAAMAAAAAAAEEAAMDBAADBAAA"
    "BAAAAAADAAABAwADAAAAAAMAAAQAAQIAAAAAAwICAAIAAAIAAAAAAQAAAAICAAMAAAEAAgQAAAAA"
    "AAQAAAAABAAAAAEAAAIAAAAAAAAAAAAAAAMABAAAAAADAgAAAAAABAAABAAAAwICAAIAAAACBAAD"
    "AAAAAAADAAABAAAAAQAAAAACAgAEAAAAAAAEBAAAAAAAAAIABAQBAAAAAAAEAQAAAAIAAQADAAAD"
    "BAADAAAEBAQAAAACAAAEAAAEAAAEAAIBAAAAAgECAAAAAAMCAAIEAgADAAMAAAADAAEAAQAAAAAB"
    "BAADAQAAAAAAAQADAAAEBAIAAAIAAQIDAAACAwAAAAMAAAAAAAAAAAQABAMAAAIDAAABAgEAAAAB"
    "AAEBAAIEAwAABAACAAQAAwEAAAAAAAAAAAABAQAAAAMBBAMAAwQABAMABAAAAwMDAQQEAAABAAEB"
    "BAAAAAAAAAABAAEDAQQAAAAABAICAAIEAAMAAAAAAwADAAQDAAECAQAAAAAAAAAAAAMCAgAAAAIA"
    "AAQEAAAAAAEAAAAAAgEAAQQAAAAEBAQDBAICAAADAgIAAQAAAQABAgQCAAABAwAAAwABAAQDAAAA"
    "AAAEAAAAAgABAAAABAAABAAAAAAAAwAEAAAAAAMAAwAAAAAAAAABAAAAAwMAAQMAAAAAAgABAAAA"
    "AAMAAQAAAQACBAAAAQAAAAECAgMAAAAAAAMAAAAEAgAAAwQCAAIAAAIAAAAAAAADBAAAAQAAAAAA"
    "AAEEAAAAAAAAAgQAAAADAAADAAAAAAAAAAAAAAIBAAEEBAAAAAAEAAAAAwABAAIBAwAAAAMEAAAA"
    "AgIDBAMAAAABAAEAAAMBAAMCAAAAAAADAAIBAAADAAAAAAABAQAAAAIAAAAEAAEAAAAAAAAABAAE"
    "AAAAAAMAAgEAAQMAAAAAAAACAAMBAgABAwAAAAAEBAAAAQADAAEAAAMBAAAAAQIAAwABAgECAQMA"
    "AAAAAAACAAAAAAEAAAAAAAAEAAAAAAMEAwABAAAEAAAAAAAAAAECAQEAAAAAAAAAAAACAAAAAQAE"
    "AAQAAAACAAQAAAAAAAAAAAEAAAABAAQBAwIAAAAAAAQCAAEBAAIAAgAAAAMEAAAEAAACAQEAAAAA"
    "AAAAAAQAAQQCAAQEAgMDAAQAAAMAAAADAAAEAAEAAwAEBAQDAAACAAEAAAAABAMDAAMAAAEAAAQA"
    "AgMAAwAABAABAAIDAAQAAAICAAIAAAAAAAIEAgAAAgAEAwIAAAABAAAEAQAAAwAAAAACBAECAQAA"
    "AwAAAwQAAwQDAAAAAAACAQQDAAAAAAAEAAAAAwMBAAAAAAQAAAAAAgIAAAADBAADBAAEAAQABAAA"
    "BAAAAwQBAAAAAAACAAACAAIAAAAEAAEABAAAAgAAAAAAAAAAAAEEAAAAAwAAAQIAAAMAAQACAwQE"
    "AQABAwAAAAAAAAAAAAMBAAAABAIAAAAAAAIEAAAAAgAAAwAEAwADAAACAAEDAwQEAwAAAAAAAAAD"
    "AwACAAIDBAAABAAEAAAAAAACAgACAgICAAAAAAAAAAADAAIDAAQBAAMAAgAAAgAAAAAAAAAAAQAE"
    "AwQAAQAAAAIBAgAAAAEAAAQAAAAAAAIAAAABAQAAAwABBAADAwABAAIAAAAAAQQBAgIABAAAAAQC"
    "AAACAgMCAwQDAAAAAAACAAABAAICAAAAAgIAAAAAAQIAAAAAAAABAAAAAAAAAAAAAAIBBAQEAAQA"
    "AgQBAAEAAAAAAAAEAwAAAAAABAAAAQABAAAAAgAAAAEAAAMBAgMAAQAAAQAAAAQAAAQAAAAAAAAA"
    "AAEAAgIAAAIAAAAAAAAEAgAAAAIBAAAAAAAAAAIEAAAAAgIAAAQAAAAAAwAAAgIAAAIABAMAAQAA"
    "AAAAAAADAAAAAAAAAAADAQADBAAAAwAAAAAAAAABBAACAQAAAAABAgADAAAAAAAAAgADAAMAAAID"
    "AAIAAAAEAAAABAAAAAAAAwABAQECAwAAAAEAAAAAAAQAAAAAAAEEAAMAAAAEAAAAAAIAAwECAAAA"
    "AQAAAAABAAAAAAAABAAAAAQABAECAAIBAAECAAAAAAADAAACAgAEAAQAAAAAAAMABAAAAQEABAAA"
    "BAEAAwMEAAMAAAQABAQDBAAAAAAAAwAAAgEEAAABAAAAAAAAAAIDAgAEAQABAwACAAAEAQQEAAIA"
    "AAADAAABAgMEBAAAAAAAAgACAAAABAQAAAABAAAAAAMDAwEAAAAEAAMABAAEAwIAAAQAAQAEAAAA"
    "AgAAAAAAAAEAAAAAAAAAAwEAAAEAAgACAAAAAQADAAAAAAEAAAAAAAAABAECAAAAAAIAAAQBAgIA"
    "AwAAAAIAAAMAAAAEAAIAAAIAAQACAAAAAAAAAAAAAAMCAAADAAEBAgAAAwAAAwADAwADAAQAAAAA"
    "AAIBAwAAAQAAAAEAAAABAAAAAAAEAAEAAAQAAgQDAgEEAgMCBAAAAQIAAgAAAgIAAAABAAQAAAAA"
    "AAAAAAEAAAAAAwQAAAAAAwAEAAAAAAADAAAAAAAEAAABBAAAAAAAAwQEAAAAAgQAAAAEAgAAAAAA"
    "AAEAAAECAAAABAIEAAAAAgAAAAECAgAAAAMDAgAAAAIBAAAEAAAAAAAAAAQAAAMAAAAAAwAAAQQA"
    "AAEDAQADAAMAAAAAAAAAAAEAAAIEAAICAQAAAAIAAAAAAAEBAAEAAAAAAAACAAMDAAEAAQAAAAAA"
    "AAADAAADAAAAAAEBAwMBAwEAAAIBAAQAAAAAAAADAAAAAAEAAAMAAAABAwMAAAAAAwAABAAAAAAA"
    "AwIAAAIDBAAEAAAAAwIAAgAAAAAAAAAAAAIAAAAAAwADAAMABAMAAgQAAwAAAwAAAAAEAgADAQAE"
    "AAQAAgAEAAAAAAADAAMAAAADAgACAQQAAAAEAAEABAAAAwEABAABAgAEBAABAwMEAAAAAQAEAgEE"
    "AAMBAAAAAAAAAAAEAAAAAAEAAAABAAAAAwAAAQIAAAMAAAAAAAAAAAAAAAACAAACBAACAAAAAAIA"
    "AAICAAEAAQAAAwMAAwEBAwAEAAMDAAQCAAIEAAABBAABBAEEAAECAQMEAAAAAAACAwADBAIBAwAB"
    "AAAAAwACAgMCAAMAAAAAAwMAAAQAAAQAAQAAAAAAAAMABAQAAwAAAAEAAgABAAAABAEAAAAAAAAC"
    "AQIAAAAAAAMAAwIAAQACAQMEAwQAAAAEAAMAAQAAAAADAQABAAQAAAABAQMBAAAEAQAAAAAAAAAE"
    "AAAAAAIEAAAEAAAAAAAEAwEAAAAAAAIAAgAAAwEAAAEAAgAAAAMAAAQEAwAAAAADAQABAwAAAAAB"
    "AwADBAAEAQAAAwAABAAABAAAAAAAAAABAAAAAAMCAAAAAgEAAAQDAQAAAAMDAAAEAAIABAAAAAAA"
    "AQMEAAAAAAAAAAAAAAEEBAAEAAQDAAAAAAAAAgAAAAMAAwAAAAEAAAAAAgAAAQAAAgAEAAADBAAA"
    "AwABAAAAAwADAAICAAIAAAICAgMEAgAAAAAAAQACAAQBBAAAAQEBAAAAAAIAAAAAAgACAAIAAAAA"
    "AQAABAIDAAAAAAAAAAAAAAAEAAAAAAABAQAAAAAEAAAAAwABAwAAAAIEAAAABAEAAgMCAwACAAAC"
    "AAADAAAAAwAAAAMAAwMAAgACAAAAAAEDBAQAAwIDAAAAAAQCAgADAAADAgAAAAAAAwAAAAMBAQEA"
    "AwEAAwABAAAAAAMCAAAAAAADAAAABAQDBAAABAEAAwAAAAQEAAAAAwAAAgIBBAACAAABAAQAAAAD"
    "AAQABAICAAAEAQMAAAACBAEAAAIAAAMEAAAABAADAAAAAAIAAAMAAQAAAAABAAIAAAACAwMDAAAA"
    "AgACAAIEAAAAAAEEAAEAAAMDAAQEBAEAAAAAAAAAAAEAAgAEAAQAAAAEAAMABAABAQMAAQADAAID"
    "AAAAAAMCAgEAAwQAAgIAAAAEAAEAAAAAAAAABAAAAAAAAAQAAAAEAAAABAAAAAAAAAAAAAAAAAAA"
    "AAAEAwMAAQMAAwQAAQABAwACAAMAAAAAAAADAQAEAgAAAgIBAAQBBAAAAAAAAAQAAQAEAgAEAAIC"
    "AAIEAAIAAgAAAAADAAAABAQAAAACBAEEAwIABAACAAAAAAMABAABAAAAAAMAAAQAAAABAAMAAAAA"
    "AgACAAMAAAAAAwAAAAIAAAAAAAAAAAMEAAQEAAIAAQAAAAQDBAAAAAQABAMAAQQAAQAAAAEEAAMD"
    "AQAABAADAAAAAAABAgAAAAAABAIAAAABAAAABAABAgECAwMAAAACAgEABAABAAAAAgEBAAAEBAAC"
    "AAAAAgEAAAMAAAACAAAAAgMAAAAAAAQBAAAAAAACAQMCAAABAAADAAADAwABAAIAAAADAAADAQAA"
    "AAAABAACAAAAAAIAAAAABAMDBAQAAAAAAAQBAAQAAAAAAAAAAQAAAAEEAAMABAEAAAAEAgAAAAMA"
    "AAAAAgMCAgIAAAAAAgAAAAAAAAMAAAAAAAEAAAAAAgMBAAMAAAAABAMEAAQAAAMAAwACBAAEAAAB"
    "AAAAAAACBAQABAAEAgQAAAAEAQMDAAMAAAIEAQAEBAADAQIABAEDAAAAAgQABAADAAAAAgACBAMB"
    "AAMDAAAAAAAAAAIDAAAAAAIABAADAAAAAQAAAAAAAAAEAQAAAgABAAMDBAIBAAAABAADAAMEAwQA"
    "AAQCAAEAAwMAAAQBAAACAAABAAEAAAQCBAMBAgAAAAAAAAAABAQCAwMABAAAAAAAAAAAAAAAAQME"
    "AAAAAQAABAACAAMCAwEBAAACAgAAAgEAAAADAAAEBAAAAAAAAAABAAABAwMAAAMCAwAEAwIAAAQA"
    "BAICAAEBAAIAAAACAgIBAAAAAgQCAgAAAQQAAAAAAAAAAAMEAAADAwQABAACBAQAAwQAAQEDAQAA"
    "BAAAAAAAAwAAAAACAAMAAgMEAwEAAAAAAAEDAAAAAAIBAAQAAAMAAAMABAAEAAEEAwMAAAABBAAE"
    "AAIEAwAAAAAAAAMAAgQAAAMAAAEAAQIAAAMDBAAABAAAAAMAAAAEAAAEAAMAAAAAAAAAAAMAAAAE"
    "AAABAwAAAQAAAAEEAAAAAAIAAQAEAAAAAAADAAMAAAQDAAAAAgQCAgEAAAIBAAAAAAADBAIAAAMA"
    "AAQAAQQAAAACAAAAAAMAAgAAAQMAAAAAAQADAAIAAAAAAgAABAAAAAQEBAAEAQQAAwABAAACAAAA"
    "AAAAAAAAAAADAAAEAAABAgADAAIAAgEDAAADAAAAAAADAwQAAAMBAAAAAAAAAAAAAgABAQADAQQA"
    "BAAAAwAAAAABAAAAAAIDAAAAAwAEAAAAAQAAAAAAAwAAAAIDAAAAAwADAAQAAAEAAAECAAIABAAA"
    "BAAABAACAAMAAQAAAAIAAgIAAgAAAAQAAQACAAACAAABAAEBAAIDAAIABAAAAwEAAgMAAAAAAAMA"
    "BAACBAAAAAAABAABBAAEAAAAAQQAAQAAAAAEAgAAAAAAAwADAAAAAAAAAAMAAAAAAAEAAAAABAEA"
    "AAAEAgIAAAIAAAAAAAAAAAAAAAEEAAADAAAAAAEAAwAAAAMEAgAAAAAAAAAAAAIEAAEAAQAABAAA"
    "BAEAAAQAAwAAAwABAAIDAwQEAAAAAwQAAAQABAMAAAECAgACAAIDAAAAAQIEAAQABAQDAAAAAAAA"
    "AAAAAAAAAwABAwAAAQADAwIAAAAAAQABAAAAAAEABAQBAwABAAADAgAEAAIAAAMABAEAAAEAAQAA"
    "BAMAAwQCAwMAAQMCAwQAAwAAAAEABAAAAAEAAgEAAAAAAAAAAAAAAAAAAgAEAQAAAAEAAAAEAwAA"
    "AQIABAMEAAABAAMAAgEEAAIAAAEEAAABAAABAQAAAAAAAgIAAAAAAAADAgABBAMEAgACBAACBAQA"
    "AgADAAACAgQAAwADAwAEBAQAAAEBAAAABAECAAAAAAAABAACAAAEBAAAAAADAAAEAAMAAAIBAAAA"
    "AAQAAQAABAAAAAACAAEDAwAEBAAAAAAAAAACAQAAAAAEAAIAAAADAAAAAAIAAwAAAAEEBAAAAgAD"
    "AAAAAgEAAAQAAAEAAAAAAAIEAAMAAwQABAACAAEBAAEAAAEABAAAAAICBAQAAQAAAgIEAAAAAAAA"
    "AAAAAAAABAIBAAAAAgIAAAACAQAAAAABAAAAAAQEAgAEAAABAAAAAAAAAAEAAAMCAwAEBAMDBAAA"
    "AAABAAABAAEBAAABAwAAAAABAAABAwMAAAABAAMEAAAAAgAAAAQAAAACAAMAAAAAAAAAAAQAAAQD"
    "AAAABAABAAIAAAIAAAAAAAICAwACAwABAAAAAAQAAwADAgAAAAAAAgEABAIAAAAAAAABBAAAAAIC"
    "AAQAAAQAAAEAAwMDAAAAAQAEBAAAAAEAAAEBAAAAAgAAAwIABAADAAAEAgAAAAAABAAAAAAAAAAC"
    "AAQAAgAEAwAAAAAEAAMEBAEAAQACAAAEAAAABAAAAAAAAAAEAQQAAAQEAAQAAgAAAQEAAQAAAAQE"
    "AAABAAAAAAQABAAEAQAABAACAwACBAQEAAAAAQEAAQABAAAAAAAAAAAAAQAAAQAAAAAEAAACAAAA"
    "BAACAAEAAAAAAAMAAAIAAAMEAQAAAAIBAAIBAAAABAECAAAAAAAAAAABAAMBAAAAAwQAAgAAAwAA"
    "AwAEAQQAAwAAAQQAAwQAAAABAAABAAAEAAQAAAACAAABAAAAAAAAAQIAAAABAAAAAAICAAACAAIA"
    "AAADAgMCAAABAAAAAwACAAMABAAAAAAAAAAAAAAAAAIAAAAAAAQBAAAAAAECAQMBAAAAAAACAAAD"
    "AAAAAAQCAAQBAAACAAAAAAMAAwIAAgMAAAABAwMDBAAABAAEAAAAAAEBAAQCAQAEAAQABAIAAAID"
    "AAEAAQAAAAACAAQAAAABAAADAQECAAAAAAQAAAMABAACAAAAAAQAAAAAAAAAAQEDAAABAwQDAwIA"
    "BAAAAQADAAAAAgAEAwAABAABAQAABAABAAQAAgAAAAAAAAQAAAMBAAACBAAEAAEEAAAABAAABAAA"
    "AAAABAMDAAEBAAAAAAAEAgMAAAAEAgADAAACAgAAAAMAAAQBAQAAAQAEAgAAAAMDAAAAAAABBAAA"
    "AAAAAwQBAAIAAAABAAIAAAIABAMAAAAEAwMAAAABAAAAAwECBAAABAAAAAACAAAAAAAAAAAEAQIB"
    "AAAABAMAAAQCAwEBAgAAAAQAAQAAAAABAAAAAAIAAwACAwECAQAAAgMCAwAEAAAEAQQAAAAAAwAA"
    "AAMAAAMAAAAABAAAAAAAAAMAAAMEAAAAAAAEAAAAAAAAAAQAAwECAAQAAAAAAgAAAAAAAAAAAAAA"
    "AAAEAAADAwAAAAMCAAIAAAAAAwAAAgADAAACAAADAAAAAAMBAAEBAAECAAADAAAEAQMDBAACAAAC"
    "AAABAAACAAQAAAAAAQAAAAAAAQABAwQAAAQCAAAAAwMAAQADAAMAAAMAAAIAAAAAAAAAAAEEAAAA"
    "AAMAAAMEAAACAAAAAAMAAwIAAQMAAgIAAAIAAQAAAAAABAMAAAAAAgEAAAABAQEBAAQAAgQDAAAA"
    "BAMAAAEAAAAAAgIAAwMAAAAABAIAAAADAAECAgIAAAEBAAMBAAQAAgAAAAIAAAIAAAAAAAQEAAAD"
    "AQEEAQIDAAACAAACAAIEAAECAAAAAgMCAwACAAABAwAAAwAAAAAABAAEAAQDAAAAAAABAQEBAAAE"
    "AAAAAwAAAgAAAAADAAECAQMAAAABAAACAAAAAAAAAwMAAAIAAAIAAAEBAAIEAAAEAAAAAAAAAAMA"
    "AQQAAAMEAAMAAwMAAQAAAAAAAAMEAAQCAAIDAAMDBAQAAAAEAAEAAAMCAQACAgAAAAEDAAQAAwAA"
    "AAAAAQQAAAICBAMAAAEAAAAAAAQDAAAAAQAAAQADAAADAAAAAAAAAQAABAAAAAAAAQADAgICAQIA"
    "AAIBAAEAAwAAAAAAAAADAwAAAAAABAIAAAAAAAAEAAMABAAAAAAAAAQAAwQABAAAAAAAAAAAAwED"
    "AAMAAAAAAAAABAMAAAAAAwEAAgABAAAAAQAAAAACAAAAAAAEAQABAAABAQAAAQAAAAMAAgABAAMA"
    "AAAABAAEAQAAAAMABAAAAAEAAQAAAwQDAAACAAQEAAACAAAEBAAAAAMBAAABAAACAAAAAAQAAAAB"
    "AAADAQIBAAADAAEAAQAAAgMBAAADAAIDAAQAAAAAAQEBAQAAAgMAAAACAAAEAwABAAAAAAAEAAAD"
    "AAEEAwEAAQAAAQACAAEAAAMAAQMAAgAAAAIAAAQAAAAAAAIDAAAAAAA="
)


def _rank_to_b64():
    """(debug helper) regenerate _RANK_B64 with jax on CPU."""
    import jax
    import jax.numpy as jnp

    cpu = jax.devices("cpu")[0]
    with jax.default_device(cpu):
        k1, k2 = jax.random.split(jax.random.key(1))
        coin = jax.random.uniform(k1, (B,)) < 0.5
        rank = jnp.where(coin, 0, jax.random.randint(k2, (B,), 0, 5))
    return base64.b64encode(np.asarray(rank, dtype=np.uint8).tobytes()).decode()


_RANK_CACHE = None


def _get_rank() -> np.ndarray:
    """rank[i]: which of the 5 nearest negatives the reference picks per row.

    Must reproduce the reference's jax.random draws bit-exactly. The default
    PRNG impl here is "rbg", whose output is backend-dependent, so compute on
    the CPU backend (the grading reference runs on CPU). Falls back to the
    embedded constant (generated the same way) if jax is unavailable.
    """
    global _RANK_CACHE
    if _RANK_CACHE is not None:
        return _RANK_CACHE
    try:
        import jax
        import jax.numpy as jnp

        cpu = jax.devices("cpu")[0]
        with jax.default_device(cpu):
            k1, k2 = jax.random.split(jax.random.key(1))
            coin = jax.random.uniform(k1, (B,)) < 0.5
            rank = jnp.where(coin, 0, jax.random.randint(k2, (B,), 0, 5))
            r = np.asarray(jax.device_get(rank)).astype(np.uint8)
    except Exception:
        r = np.frombuffer(base64.b64decode(_RANK_B64), dtype=np.uint8)
    assert r.shape == (B,)
    _RANK_CACHE = r
    return r


_NC_CACHE = None


def _build_nc():
    import os as _os

    kparts = int(_os.environ.get("K_PARTS", "63"))
    # bits: 1=norms(np2/na2+sqrt+recip) 2=gpsimd scales 4=ttr rawii
    #       8=transposes+evicts 16=main loop 32=epilogue
    import concourse.mybir as mybir
    import concourse.tile as tile
    from concourse import bacc
    from concourse.masks import make_identity

    F32 = mybir.dt.float32
    F32R = mybir.dt.float32r
    AF = mybir.ActivationFunctionType

    nc = bacc.Bacc()
    xa = nc.dram_tensor("xa", [M, D], F32, kind="ExternalInput").ap()
    xp = nc.dram_tensor("xp", [B, D], F32, kind="ExternalInput").ap()
    oh = nc.dram_tensor("oh", [M, 8], F32, kind="ExternalInput").ap()
    loss = nc.dram_tensor("loss", [128, RB], F32, kind="ExternalOutput").ap()

    with tile.TileContext(nc) as tc:
        with (
            tc.tile_pool(name="const", bufs=1) as constp,
            tc.tile_pool(name="big", bufs=1) as bigp,
            tc.tile_pool(name="stage", bufs=3) as stagep,
            tc.tile_pool(name="scaled", bufs=2) as scaledp,
            tc.tile_pool(name="small", bufs=4) as smallp,
            tc.tile_pool(name="cand", bufs=RB) as candp,
            tc.tile_pool(name="pst", bufs=2, space="PSUM") as pst,
            tc.tile_pool(name="psg", bufs=2, space="PSUM") as psg,
        ):
            # ---------------- constants ----------------
            ident = constp.tile([128, 128], F32)
            make_identity(nc, ident)
            negid_f = constp.tile([128, 128], F32)
            nc.gpsimd.memset(negid_f, 0.0)
            nc.gpsimd.affine_select(
                out=negid_f,
                in_=negid_f,
                compare_op=mybir.AluOpType.not_equal,
                fill=NEG_BIG,
                base=0,
                pattern=[[-1, 128]],
                channel_multiplier=1,
            )
            # fp32r copies (gpsimd ISA ops cannot write f32r directly)
            negid = constp.tile([128, 128], F32R)
            nc.scalar.copy(negid, negid_f)
            identr = constp.tile([128, 128], F32R)
            nc.scalar.copy(identr, ident)

            oh_sb = constp.tile([128, RB * 8], F32)
            nc.sync.dma_start(
                oh_sb.rearrange("p (r k) -> p r k", r=RB),
                oh.rearrange("(r p) k -> p r k", p=128),
            )

            # ---------------- anchor slab prep ----------------
            xa_res = bigp.tile([128, RB * D], F32)
            nc.sync.dma_start(
                xa_res.rearrange("p (r d) -> p r d", r=RB),
                xa.rearrange("(r p) d -> p r d", p=128),
            )
            sq_scr = smallp.tile([128, D], F32, tag="sqscr")
            na2 = constp.tile([128, RB], F32)
            na_half = constp.tile([128, RB], F32)
            inv2na = constp.tile([128, RB], F32)
            if kparts & 1:
                for r in range(RB):
                    nc.scalar.activation(
                        sq_scr,
                        xa_res[:, r * D : (r + 1) * D],
                        AF.Square,
                        accum_out=na2[:, r : r + 1],
                    )
                # na2 -> 2/na:  1/sqrt(na2/4)
                nc.scalar.activation(na_half, na2, AF.Sqrt, scale=0.25)
                nc.vector.reciprocal(inv2na, na_half)
            else:
                nc.vector.memset(na2, 1.0)
                nc.vector.memset(na_half, 1.0)
                nc.vector.memset(inv2na, 1.0)

            xa_s = bigp.tile([128, RB * D], F32)
            if kparts & 2:
                for r in range(RB):
                    nc.gpsimd.tensor_scalar_mul(
                        xa_s[:, r * D : (r + 1) * D],
                        xa_res[:, r * D : (r + 1) * D],
                        inv2na[:, r : r + 1],
                    )
            else:
                nc.vector.tensor_copy(xa_s, xa_res)

            # aT[k] = transposed scaled anchors, K-chunk k: [128, M] fp32r
            aT = [bigp.tile([128, M], F32R, tag=f"aT{k}", name=f"aT{k}") for k in range(2)]
            for k in range(2 if kparts & 8 else 0):
                for r4 in range(RB // 4):  # groups of 4 row blocks
                    ptile = pst.tile([128, 512], F32)
                    for j in range(4):
                        r = r4 * 4 + j
                        nc.tensor.transpose(
                            ptile[:, j * 128 : (j + 1) * 128],
                            xa_s[:, r * D + k * 128 : r * D + k * 128 + 128],
                            ident,
                        )
                    nc.scalar.copy(aT[k][:, r4 * 512 : (r4 + 1) * 512], ptile)

            # ---------------- local-positive head (for s_ii row dots) -------
            xp_head = bigp.tile([128, RB * D], F32)
            nc.sync.dma_start(
                xp_head.rearrange("p (r d) -> p r d", r=RB),
                xp[:M].rearrange("(r p) d -> p r d", p=128),
            )
            rawii = constp.tile([128, RB], F32)
            if not kparts & 4:
                nc.vector.memset(rawii, 0.0)
            for r in range(RB if kparts & 4 else 0):
                # tensor_tensor_reduce (DVE ISA op) crashes this runtime's
                # TRN2 exec unit -- use mul (GpSimd) + reduce_sum (DVE).
                dot_scr = smallp.tile([128, D], F32, tag="dotscr")
                nc.gpsimd.tensor_mul(
                    dot_scr,
                    xa_res[:, r * D : (r + 1) * D],
                    xp_head[:, r * D : (r + 1) * D],
                )
                nc.vector.reduce_sum(
                    rawii[:, r : r + 1], dot_scr, axis=mybir.AxisListType.X
                )

            # ---------------- positives: chunked norm+scale+transpose -------
            pT = [bigp.tile([128, B], F32R, tag=f"pT{k}", name=f"pT{k}") for k in range(2)]
            np2 = constp.tile([128, CH * 4], F32)
            nps = constp.tile([128, CH * 4], F32)
            invnp = constp.tile([128, CH * 4], F32)
            for c in range(CH):
                stage = stagep.tile([128, CW // 128 * D], F32, tag="xpstage")
                nc.sync.dma_start(
                    stage.rearrange("p (s d) -> p s d", s=CW // 128),
                    xp[c * CW : (c + 1) * CW].rearrange("(s p) d -> p s d", p=128),
                )
                sq2 = smallp.tile([128, D], F32, tag="sqscr2")
                if kparts & 1:
                    for s in range(CW // 128):
                        b = c * 4 + s
                        nc.scalar.activation(
                            sq2,
                            stage[:, s * D : (s + 1) * D],
                            AF.Square,
                            accum_out=np2[:, b : b + 1],
                        )
                    nc.scalar.activation(
                        nps[:, c * 4 : (c + 1) * 4],
                        np2[:, c * 4 : (c + 1) * 4],
                        AF.Sqrt,
                    )
                    nc.vector.reciprocal(
                        invnp[:, c * 4 : (c + 1) * 4], nps[:, c * 4 : (c + 1) * 4]
                    )
                else:
                    nc.vector.memset(invnp[:, c * 4 : (c + 1) * 4], 1.0)
                xps = scaledp.tile([128, CW // 128 * D], F32, tag="xps")
                if kparts & 2:
                    for s in range(CW // 128):
                        b = c * 4 + s
                        nc.gpsimd.tensor_scalar_mul(
                            xps[:, s * D : (s + 1) * D],
                            stage[:, s * D : (s + 1) * D],
                            invnp[:, b : b + 1],
                        )
                else:
                    nc.vector.tensor_copy(xps, stage)
                for k in range(2 if kparts & 8 else 0):
                    ptile = pst.tile([128, 512], F32)
                    for s in range(CW // 128):
                        nc.tensor.transpose(
                            ptile[:, s * 128 : (s + 1) * 128],
                            xps[:, s * D + k * 128 : s * D + k * 128 + 128],
                            ident,
                        )
                    nc.scalar.copy(pT[k][:, c * CW : (c + 1) * CW], ptile)

            # ---------------- main loop: matmul granules + top-8 ------------
            cand = [
                candp.tile([128, NG * 16], F32, tag=f"cand{r}", name=f"cand{r}")
                for r in range(RB)
            ]
            for g in range(NG if kparts & 16 else 0):
                for r in range(RB):
                    gt = psg.tile([128, GW], F32)
                    dh = r // 4 if g == 0 else -1  # bank holding the diagonal
                    for h in range(2):
                        for k in range(2):
                            nc.tensor.matmul(
                                gt[:, h * 512 : (h + 1) * 512],
                                aT[k][:, r * 128 : (r + 1) * 128],
                                pT[k][:, g * GW + h * 512 : g * GW + (h + 1) * 512],
                                start=(k == 0),
                                stop=(k == 1 and h != dh),
                            )
                        if h == dh:
                            # rotated layout: row block r's self-cols are
                            # [r*128, r*128+128) of granule 0 on every core.
                            # Accumulate -3e38*I there via the PE itself so no
                            # other engine ever writes PSUM.
                            nc.tensor.matmul(
                                gt[:, r * 128 : r * 128 + 128],
                                negid,
                                identr,
                                start=False,
                                stop=True,
                            )
                    for h in range(2):
                        # single-bank PSUM reads for Max8
                        nc.vector.max(
                            out=cand[r][:, (2 * g + h) * 8 : (2 * g + h + 1) * 8],
                            in_=gt[:, h * 512 : (h + 1) * 512],
                        )

            # ---------------- epilogue: merge, select, loss -----------------
            loss_sb = constp.tile([128, RB], F32)
            sii = constp.tile([128, RB], F32)
            sel_scr = smallp.tile([128, 8], F32, tag="selscr")
            if not kparts & 16:
                for r in range(RB):
                    nc.vector.max(out=cand[r][:, 0:8], in_=xa_res[:, 0:512])
            for r in range(RB if kparts & 32 else 0):
                # s_ii = rawii * (2/na) * (1/np)  (local rows = first RB blocks)
                nc.vector.tensor_scalar(
                    sii[:, r : r + 1],
                    rawii[:, r : r + 1],
                    inv2na[:, r : r + 1],
                    invnp[:, r : r + 1],
                    op0=mybir.AluOpType.mult,
                    op1=mybir.AluOpType.mult,
                )
                top8 = smallp.tile([128, 8], F32, tag="top8")
                nc.vector.max(out=top8, in_=cand[r])
                selv = smallp.tile([128, 1], F32, tag="selv")
                nc.vector.tensor_mul(sel_scr, top8, oh_sb[:, r * 8 : (r + 1) * 8])
                nc.vector.reduce_sum(selv, sel_scr, axis=mybir.AxisListType.X)
                nc.vector.tensor_sub(loss_sb[:, r : r + 1], selv, sii[:, r : r + 1])
            if not kparts & 32:
                for r in range(RB):
                    nc.vector.tensor_copy(
                        loss_sb[:, r : r + 1], cand[r][:, 0:1]
                    )
            relu_sb = constp.tile([128, RB], F32)
            nc.scalar.activation(relu_sb, loss_sb, AF.Relu)
            nc.sync.dma_start(loss, relu_sb)

    nc.compile()
    return nc


def _get_nc():
    global _NC_CACHE
    if _NC_CACHE is None:
        _NC_CACHE = _build_nc()
    return _NC_CACHE


def kernel(x: np.ndarray, _want_timing: bool = False):
    """x: [8192, 2, 256] float32 -> scalar float32 loss (0-d ndarray)."""
    from concourse.bass_utils import run_bass_kernel_spmd

    x = np.ascontiguousarray(np.asarray(x, dtype=np.float32))
    assert x.shape == (B, 2, D)
    x0 = x[:, 0, :]
    x1 = np.ascontiguousarray(x[:, 1, :])

    rank = _get_rank()
    onehot = np.zeros((B, 8), dtype=np.float32)
    onehot[np.arange(B), rank] = 1.0

    in_maps = []
    for c in range(NCORES):
        lo = c * M
        in_maps.append(
            {
                "xa": np.ascontiguousarray(x0[lo : lo + M]),
                "xp": np.ascontiguousarray(np.roll(x1, -lo, axis=0)),
                "oh": np.ascontiguousarray(onehot[lo : lo + M]),
            }
        )

    nc = _get_nc()
    res = run_bass_kernel_spmd(nc, in_maps, list(range(NCORES)))
    per_row = np.concatenate(
        [res.results[c]["loss"].T.reshape(M) for c in range(NCORES)]
    )  # loss[p, r] -> row r*128+p; .T gives [r, p] -> flat local rows
    out = np.float32(np.mean(per_row))
    if _want_timing:
        return np.asarray(out), res, per_row
    return np.asarray(out)


if __name__ == "__main__":
    rng = np.random.default_rng(0)
    x = rng.standard_normal((B, 2, D)).astype(np.float32)
    print(kernel(x))



# revision 24
# speedup vs baseline: 1.7256x; 1.7256x over previous
"""Trainium2 Bass kernel for nn_LossFunction_103079215159 (triplet-style loss
with online hard-negative mining).

Math (B=8192 rows, D=256 features; x[:,0]=anchors x0, x[:,1]=positives x1):
  a = l2norm(x0), p = l2norm(x1)
  dist[i,j] = || a_i - p_j + eps ||  (via gemm expansion), diag masked +inf
  top5 smallest per row -> pick rank[i]-th (RNG-derived, data-independent)
  loss = mean relu(||a_i-p_i+eps||^2 - ||a_i-p_neg+eps||^2)

Reduction: with s[i,j] = <2*x0_i/||x0_i||, x1_j/||x1_j||> = 2*cos_ij,
  loss_i = relu(s_sel[i] - s_ii[i]) where s_sel is the rank[i]-th LARGEST
  masked s row value (eps terms cancel/negligible; validated rel err ~1e-6
  in the fp32 pipeline, ~1e-4 with the bf16 operand rounding used here).

Distribution: 8-way data parallel over anchor rows; positives matrix rotated
per core (np.roll) so the self-match diagonal lands at identical local
addresses (single SPMD program).

v3 design (per core), tuned against the TimelineSim cost model (118 us
modeled vs 141 us for the fp32r/Max8-only baseline):
  - operand prep: ACT square+accum row norms; DVE reciprocal; scale-and-
    cast rows to bf16 (DVE for the critical first chunks / anchors, Pool
    for the rest). Transposed [K=2x128, rows] bf16 matmul operands are
    built two ways: granules 0-1 and the anchors via PE transpose + DVE
    PSUM evict (PE/DVE are idle during fill), granules 2-7 via DRAM
    writeback + XBAR transposing DMA (cheap, off the critical path).
    DMA issue order is planned explicitly (in-order queues head-of-line
    block on waits): loads stream on SP ahead of the transposes.
  - main loop over 64 units (granule g x row-block r), PSUM [128,1024]
    f32: 4 bf16 matmuls + PE-side -3e38*I diag-mask accumulate; per unit:
      path A  (granules not in P2 set): DVE Max8 straight from PSUM
              (1024 wide) -> 8 bf16 candidates
      path P2: ACT copy-convert PSUM->SBUF bf16; DVE tensor_max folds
              8:1 -> 128 bf16 group-max candidates (two of a row's top-5
              share a group of 8 with p~0.9% -> rel loss error ~1e-4).
      (DVE cannot read two PSUM operands in one instruction, so folding
      directly from PSUM is impossible; the A/P2 split balances DVE vs
      ACT busy time, with P2 scheduled late so it lands after ACT's
      norm squares drain.)
  - per row-block: top8s of granules 0-6 premerge during granule 7;
    final Max8, one-hot select of the rank-th value, s_ii from Pool
    row-dot + DVE reduce, relu -> per-row losses -> DRAM.
Host: input slicing/rotation, RNG one-hot (embedded constant), final mean.
"""

import base64
import os

import numpy as np

B = 8192
D = 256
NCORES = 8
M = B // NCORES  # 1024 anchor rows per core
RB = M // 128  # 8 row blocks per core
NG = 8  # granules of 1024 cols each
GW = 1024  # granule width
CH = 16  # x1 chunks of 512 rows
CW = 512  # chunk rows

NEG_BIG = -3.0e38

# Per-(granule, row-block) scan path: number of P2 (ACT convert + DVE fold)
# units per row-block across its 8 granules; the rest are P1 (all-DVE
# fold-from-PSUM) units.
N_P2_PER_RB = int(os.environ.get("N_P2_PER_RB", "3"))

# rank[i] in {0..4}: which of the 5 nearest negatives to use per row.
# Reproduces exactly (verified):
#   k1, k2 = jax.random.split(jax.random.key(1))
#   coin = jax.random.uniform(k1, (8192,)) < 0.5
#   rank = jnp.where(coin, 0, jax.random.randint(k2, (8192,), 0, 5))
_RANK_B64 = (
    "AAIEAAAAAAAAAAIAAwAAAAAAAAAAAAMAAAIAAAMABAAAAAAAAwACAAABAAQCBAADAAACAgAEAwAC"
    "AAMEAAAAAwEEAQMAAAIAAgAAAAAAAAAEAAQAAwAABAECAAIAAAAAAgADAAACAwQABAAAAgMAAgAE"
    "AwAAAgACAAECAAEAAAECAQEBAAAABAACBAAAAAAAAAEAAAAEAQAAAAIAAgADAAEAAAAAAQAAAQME"
    "AgAAAAEEAAAAAAMAAQAAAAAEAAAEAQAAAAAAAAAAAAAAAAADAQQAAAAAAgABAAAAAAADAAADAAQA"
    "AAAAAwMAAAAEAAAAAAAAAAEAAAMAAAAAAAQAAAACAgAEAQAAAAABAAADAgABAAIAAAAAAwQCAAAD"
    "AgAAAAADAgAAAQAABAAABAAAAAAAAAIAAAEABAADAAAAAAAEAAAAAQEBAAAAAAMAAAIAAAAAAAMA"
    "AwIDAAEAAQQAAAIAAAEEAAECAAIAAAEAAAADAAIAAQICAAABAgAAAQAAAAIAAAADAAEDBAAAAQEA"
    "AgAAAAAEBAAAAAEAAgECAAIEAAAABAAEAQIABAAAAAAAAAAAAAMBAQAAAAMCAgADAAIDAwQDBAAE"
    "AAAAAAAAAAEAAAEAAwMAAAAAAAAAAAABAAAAAAAAAAEAAAADAgMAAAMAAAAAAAMAAQAAAAAAAgAA"
    "BAAAAAMBAQABAAAAAAAAAAIAAwAAAgAEAwABAAAAAAAAAAAAAAIAAgABAgAEAAABAQIAAgIDAgAE"
    "AAAAAAAAAQAABAAEAAAAAAAAAQIAAgAAAAMAAQACAAAAAAADAAQAAQABBAAEAAMABAABAQADAQAA"
    "AgABAgAEAAIAAAAAAgAAAwAAAwAAAAAEAAAAAAEAAAAAAAIEAAAAAgAABAEAAgAAAAAAAAEAAAAC"
    "AAECBAADAAAAAQAAAAIAAAAAAgMAAAAAAQAAAAQAAAAAAAMEAwEAAgEAAAAAAAAABAADAQIDAAAA"
    "AAEAAwAAAgAAAAEAAgAAAAAAAgAAAAAABAAEAAACAAIAAAQAAgADAAEAAAQAAAACAAECAwIEAAAA"
    "BAQAAAQABAMAAAQAAwIAAQMAAAQAAAACAAAEAAAABAAAAAAAAAMBAAEAAAQDAAAAAAQDAAAAAAIA"
    "AAAEAwACAAQAAgACAAACAQQAAAQDAgQDAQAAAAAEAAADBAECBAAEAAEBAAAAAAEAAgAAAwAAAgAB"
    "AwAAAgAEBAAAAAIEAAAAAwACAAIBAAABAwQAAQAAAAQAAAAAAAIAAAEBAAIAAAAAAAEAAAAAAAEB"
    "AAAAAgACAAAAAAMAAwAAAAAABAMABAMAAQQBAAQCAAEDAAAAAAIAAAAEAAMDAAAEAAEAAQAAAAAA"
    "AAICBAABAQQEAAAAAAQAAQABAAEEAAACBAAAAAMAAAAABAAAAAEBAAICAAIAAAAAAAAEBAAAAAMC"
    "AAQDAAABAAQCAAEAAAAABAQEAAIBAAAAAgAEAAEAAAIEBAACAAIAAAAABAMDBAQAAAAAAAIAAgAA"
    "AAACAAABAwMDAAAAAAAAAAACAQAAAwAAAAAEAAAAAAMAAAAAAgMAAAICAAMAAAAEAAAAAAABAAAA"
    "AAABAAAAAAMAAAEEAAIDAAEBAAQAAAMCAAAAAAAEAAACAAMAAAACAwAAAwAEAAAAAAQAAwABAAAC"
    "AwAAAAEABAQBAAIAAAIAAwAEAAEAAAACAgAAAAEEAAQAAAADAAMDAAQDBAABBAACAwAAAAAEAAMA"
    "AgQABAIAAAAEAAQCAQMAAAIBAAIAAAQEAAACAAEAAAAAAAEAAAABAAEAAAAABAAAAAAABAADAAAA"
    "BAABBAABAAADAAAAAAAAAAAAAQAAAAAAAAMAAQAAAQACAAAAAAACAAMAAAMAAwIBAAAABAAAAAMA"
    "AAAAAAABAAABAQIBAAAAAgAAAAAEAAAAAAQAAAAAAwAAAAAAAgAAAAAAAAAAAAACAgAAAAABBAAA"
    "AwACAAEDAAAAAAQAAQACAAAEAAAAAgAAAAIAAAMBAAAAAAIEAwAAAAQAAAMAAAMAAAAAAAAAAAMC"
    "BAQAAAMAAAEBAQAAAAAAAAIAAAMAAAMAAAAAAAIABAAAAAABAgAAAAAEAAQCAAIAAAIDAAMBAAAA"
    "AwAAAQADAwABAAADAAAEAwAAAAAABAMAAAEAAAAAAAAAAAAAAAAAAAAAAAACAAAAAAICAgACAAMA"
    "AAACAwAAAAIAAQAAAAAEAQAAAgAEAAEAAwAEAAAAAAAAAAQAAwAAAwAAAAQEAgAAAAMEAAAAAAAB"
    "AwQAAgADAgEDAAQDAAAAAAIAAAAAAAAAAAAABAQAAAEEBAABAAAAAQQAAAAABAAAAAMCAAAAAAAD"
    "BAAAAAEEAwIAAAADAAAAAAAEAAIAAAMBAAADAAAAAAAAAgAAAAMCAAAEAgACAAADAAAAAwABBAAD"
    "AAIAAAAAAQAABAADAAAAAAQAAQABAAMAAwADAAAAAAAAAAMEAwADAwQBAAAAAAMAAAAAAAEDAAAE"
    "AQAAAAAAAgAAAQAAAAICAAIEAAABBAACAAABAgAAAQAABAIDAgAEAAMAAAAAAAEEAAMDBAADBAAA"
    "BAAAAAADAAABAwADAAAAAAMAAAQAAQIAAAAAAwICAAIAAAIAAAAAAQAAAAICAAMAAAEAAgQAAAAA"
    "AAQAAAAABAAAAAEAAAIAAAAAAAAAAAAAAAMABAAAAAADAgAAAAAABAAABAAAAwICAAIAAAACBAAD"
    "AAAAAAADAAABAAAAAQAAAAACAgAEAAAAAAAEBAAAAAAAAAIABAQBAAAAAAAEAQAAAAIAAQADAAAD"
    "BAADAAAEBAQAAAACAAAEAAAEAAAEAAIBAAAAAgECAAAAAAMCAAIEAgADAAMAAAADAAEAAQAAAAAB"
    "BAADAQAAAAAAAQADAAAEBAIAAAIAAQIDAAACAwAAAAMAAAAAAAAAAAQABAMAAAIDAAABAgEAAAAB"
    "AAEBAAIEAwAABAACAAQAAwEAAAAAAAAAAAABAQAAAAMBBAMAAwQABAMABAAAAwMDAQQEAAABAAEB"
    "BAAAAAAAAAABAAEDAQQAAAAABAICAAIEAAMAAAAAAwADAAQDAAECAQAAAAAAAAAAAAMCAgAAAAIA"
    "AAQEAAAAAAEAAAAAAgEAAQQAAAAEBAQDBAICAAADAgIAAQAAAQABAgQCAAABAwAAAwABAAQDAAAA"
    "AAAEAAAAAgABAAAABAAABAAAAAAAAwAEAAAAAAMAAwAAAAAAAAABAAAAAwMAAQMAAAAAAgABAAAA"
    "AAMAAQAAAQACBAAAAQAAAAECAgMAAAAAAAMAAAAEAgAAAwQCAAIAAAIAAAAAAAADBAAAAQAAAAAA"
    "AAEEAAAAAAAAAgQAAAADAAADAAAAAAAAAAAAAAIBAAEEBAAAAAAEAAAAAwABAAIBAwAAAAMEAAAA"
    "AgIDBAMAAAABAAEAAAMBAAMCAAAAAAADAAIBAAADAAAAAAABAQAAAAIAAAAEAAEAAAAAAAAABAAE"
    "AAAAAAMAAgEAAQMAAAAAAAACAAMBAgABAwAAAAAEBAAAAQADAAEAAAMBAAAAAQIAAwABAgECAQMA"
    "AAAAAAACAAAAAAEAAAAAAAAEAAAAAAMEAwABAAAEAAAAAAAAAAECAQEAAAAAAAAAAAACAAAAAQAE"
    "AAQAAAACAAQAAAAAAAAAAAEAAAABAAQBAwIAAAAAAAQCAAEBAAIAAgAAAAMEAAAEAAACAQEAAAAA"
    "AAAAAAQAAQQCAAQEAgMDAAQAAAMAAAADAAAEAAEAAwAEBAQDAAACAAEAAAAABAMDAAMAAAEAAAQA"
    "AgMAAwAABAABAAIDAAQAAAICAAIAAAAAAAIEAgAAAgAEAwIAAAABAAAEAQAAAwAAAAACBAECAQAA"
    "AwAAAwQAAwQDAAAAAAACAQQDAAAAAAAEAAAAAwMBAAAAAAQAAAAAAgIAAAADBAADBAAEAAQABAAA"
    "BAAAAwQBAAAAAAACAAACAAIAAAAEAAEABAAAAgAAAAAAAAAAAAEEAAAAAwAAAQIAAAMAAQACAwQE"
    "AQABAwAAAAAAAAAAAAMBAAAABAIAAAAAAAIEAAAAAgAAAwAEAwADAAACAAEDAwQEAwAAAAAAAAAD"
    "AwACAAIDBAAABAAEAAAAAAACAgACAgICAAAAAAAAAAADAAIDAAQBAAMAAgAAAgAAAAAAAAAAAQAE"
    "AwQAAQAAAAIBAgAAAAEAAAQAAAAAAAIAAAABAQAAAwABBAADAwABAAIAAAAAAQQBAgIABAAAAAQC"
    "AAACAgMCAwQDAAAAAAACAAABAAICAAAAAgIAAAAAAQIAAAAAAAABAAAAAAAAAAAAAAIBBAQEAAQA"
    "AgQBAAEAAAAAAAAEAwAAAAAABAAAAQABAAAAAgAAAAEAAAMBAgMAAQAAAQAAAAQAAAQAAAAAAAAA"
    "AAEAAgIAAAIAAAAAAAAEAgAAAAIBAAAAAAAAAAIEAAAAAgIAAAQAAAAAAwAAAgIAAAIABAMAAQAA"
    "AAAAAAADAAAAAAAAAAADAQADBAAAAwAAAAAAAAABBAACAQAAAAABAgADAAAAAAAAAgADAAMAAAID"
    "AAIAAAAEAAAABAAAAAAAAwABAQECAwAAAAEAAAAAAAQAAAAAAAEEAAMAAAAEAAAAAAIAAwECAAAA"
    "AQAAAAABAAAAAAAABAAAAAQABAECAAIBAAECAAAAAAADAAACAgAEAAQAAAAAAAMABAAAAQEABAAA"
    "BAEAAwMEAAMAAAQABAQDBAAAAAAAAwAAAgEEAAABAAAAAAAAAAIDAgAEAQABAwACAAAEAQQEAAIA"
    "AAADAAABAgMEBAAAAAAAAgACAAAABAQAAAABAAAAAAMDAwEAAAAEAAMABAAEAwIAAAQAAQAEAAAA"
    "AgAAAAAAAAEAAAAAAAAAAwEAAAEAAgACAAAAAQADAAAAAAEAAAAAAAAABAECAAAAAAIAAAQBAgIA"
    "AwAAAAIAAAMAAAAEAAIAAAIAAQACAAAAAAAAAAAAAAMCAAADAAEBAgAAAwAAAwADAwADAAQAAAAA"
    "AAIBAwAAAQAAAAEAAAABAAAAAAAEAAEAAAQAAgQDAgEEAgMCBAAAAQIAAgAAAgIAAAABAAQAAAAA"
    "AAAAAAEAAAAAAwQAAAAAAwAEAAAAAAADAAAAAAAEAAABBAAAAAAAAwQEAAAAAgQAAAAEAgAAAAAA"
    "AAEAAAECAAAABAIEAAAAAgAAAAECAgAAAAMDAgAAAAIBAAAEAAAAAAAAAAQAAAMAAAAAAwAAAQQA"
    "AAEDAQADAAMAAAAAAAAAAAEAAAIEAAICAQAAAAIAAAAAAAEBAAEAAAAAAAACAAMDAAEAAQAAAAAA"
    "AAADAAADAAAAAAEBAwMBAwEAAAIBAAQAAAAAAAADAAAAAAEAAAMAAAABAwMAAAAAAwAABAAAAAAA"
    "AwIAAAIDBAAEAAAAAwIAAgAAAAAAAAAAAAIAAAAAAwADAAMABAMAAgQAAwAAAwAAAAAEAgADAQAE"
    "AAQAAgAEAAAAAAADAAMAAAADAgACAQQAAAAEAAEABAAAAwEABAABAgAEBAABAwMEAAAAAQAEAgEE"
    "AAMBAAAAAAAAAAAEAAAAAAEAAAABAAAAAwAAAQIAAAMAAAAAAAAAAAAAAAACAAACBAACAAAAAAIA"
    "AAICAAEAAQAAAwMAAwEBAwAEAAMDAAQCAAIEAAABBAABBAEEAAECAQMEAAAAAAACAwADBAIBAwAB"
    "AAAAAwACAgMCAAMAAAAAAwMAAAQAAAQAAQAAAAAAAAMABAQAAwAAAAEAAgABAAAABAEAAAAAAAAC"
    "AQIAAAAAAAMAAwIAAQACAQMEAwQAAAAEAAMAAQAAAAADAQABAAQAAAABAQMBAAAEAQAAAAAAAAAE"
    "AAAAAAIEAAAEAAAAAAAEAwEAAAAAAAIAAgAAAwEAAAEAAgAAAAMAAAQEAwAAAAADAQABAwAAAAAB"
    "AwADBAAEAQAAAwAABAAABAAAAAAAAAABAAAAAAMCAAAAAgEAAAQDAQAAAAMDAAAEAAIABAAAAAAA"
    "AQMEAAAAAAAAAAAAAAEEBAAEAAQDAAAAAAAAAgAAAAMAAwAAAAEAAAAAAgAAAQAAAgAEAAADBAAA"
    "AwABAAAAAwADAAICAAIAAAICAgMEAgAAAAAAAQACAAQBBAAAAQEBAAAAAAIAAAAAAgACAAIAAAAA"
    "AQAABAIDAAAAAAAAAAAAAAAEAAAAAAABAQAAAAAEAAAAAwABAwAAAAIEAAAABAEAAgMCAwACAAAC"
    "AAADAAAAAwAAAAMAAwMAAgACAAAAAAEDBAQAAwIDAAAAAAQCAgADAAADAgAAAAAAAwAAAAMBAQEA"
    "AwEAAwABAAAAAAMCAAAAAAADAAAABAQDBAAABAEAAwAAAAQEAAAAAwAAAgIBBAACAAABAAQAAAAD"
    "AAQABAICAAAEAQMAAAACBAEAAAIAAAMEAAAABAADAAAAAAIAAAMAAQAAAAABAAIAAAACAwMDAAAA"
    "AgACAAIEAAAAAAEEAAEAAAMDAAQEBAEAAAAAAAAAAAEAAgAEAAQAAAAEAAMABAABAQMAAQADAAID"
    "AAAAAAMCAgEAAwQAAgIAAAAEAAEAAAAAAAAABAAAAAAAAAQAAAAEAAAABAAAAAAAAAAAAAAAAAAA"
    "AAAEAwMAAQMAAwQAAQABAwACAAMAAAAAAAADAQAEAgAAAgIBAAQBBAAAAAAAAAQAAQAEAgAEAAIC"
    "AAIEAAIAAgAAAAADAAAABAQAAAACBAEEAwIABAACAAAAAAMABAABAAAAAAMAAAQAAAABAAMAAAAA"
    "AgACAAMAAAAAAwAAAAIAAAAAAAAAAAMEAAQEAAIAAQAAAAQDBAAAAAQABAMAAQQAAQAAAAEEAAMD"
    "AQAABAADAAAAAAABAgAAAAAABAIAAAABAAAABAABAgECAwMAAAACAgEABAABAAAAAgEBAAAEBAAC"
    "AAAAAgEAAAMAAAACAAAAAgMAAAAAAAQBAAAAAAACAQMCAAABAAADAAADAwABAAIAAAADAAADAQAA"
    "AAAABAACAAAAAAIAAAAABAMDBAQAAAAAAAQBAAQAAAAAAAAAAQAAAAEEAAMABAEAAAAEAgAAAAMA"
    "AAAAAgMCAgIAAAAAAgAAAAAAAAMAAAAAAAEAAAAAAgMBAAMAAAAABAMEAAQAAAMAAwACBAAEAAAB"
    "AAAAAAACBAQABAAEAgQAAAAEAQMDAAMAAAIEAQAEBAADAQIABAEDAAAAAgQABAADAAAAAgACBAMB"
    "AAMDAAAAAAAAAAIDAAAAAAIABAADAAAAAQAAAAAAAAAEAQAAAgABAAMDBAIBAAAABAADAAMEAwQA"
    "AAQCAAEAAwMAAAQBAAACAAABAAEAAAQCBAMBAgAAAAAAAAAABAQCAwMABAAAAAAAAAAAAAAAAQME"
    "AAAAAQAABAACAAMCAwEBAAACAgAAAgEAAAADAAAEBAAAAAAAAAABAAABAwMAAAMCAwAEAwIAAAQA"
    "BAICAAEBAAIAAAACAgIBAAAAAgQCAgAAAQQAAAAAAAAAAAMEAAADAwQABAACBAQAAwQAAQEDAQAA"
    "BAAAAAAAAwAAAAACAAMAAgMEAwEAAAAAAAEDAAAAAAIBAAQAAAMAAAMABAAEAAEEAwMAAAABBAAE"
    "AAIEAwAAAAAAAAMAAgQAAAMAAAEAAQIAAAMDBAAABAAAAAMAAAAEAAAEAAMAAAAAAAAAAAMAAAAE"
    "AAABAwAAAQAAAAEEAAAAAAIAAQAEAAAAAAADAAMAAAQDAAAAAgQCAgEAAAIBAAAAAAADBAIAAAMA"
    "AAQAAQQAAAACAAAAAAMAAgAAAQMAAAAAAQADAAIAAAAAAgAABAAAAAQEBAAEAQQAAwABAAACAAAA"
    "AAAAAAAAAAADAAAEAAABAgADAAIAAgEDAAADAAAAAAADAwQAAAMBAAAAAAAAAAAAAgABAQADAQQA"
    "BAAAAwAAAAABAAAAAAIDAAAAAwAEAAAAAQAAAAAAAwAAAAIDAAAAAwADAAQAAAEAAAECAAIABAAA"
    "BAAABAACAAMAAQAAAAIAAgIAAgAAAAQAAQACAAACAAABAAEBAAIDAAIABAAAAwEAAgMAAAAAAAMA"
    "BAACBAAAAAAABAABBAAEAAAAAQQAAQAAAAAEAgAAAAAAAwADAAAAAAAAAAMAAAAAAAEAAAAABAEA"
    "AAAEAgIAAAIAAAAAAAAAAAAAAAEEAAADAAAAAAEAAwAAAAMEAgAAAAAAAAAAAAIEAAEAAQAABAAA"
    "BAEAAAQAAwAAAwABAAIDAwQEAAAAAwQAAAQABAMAAAECAgACAAIDAAAAAQIEAAQABAQDAAAAAAAA"
    "AAAAAAAAAwABAwAAAQADAwIAAAAAAQABAAAAAAEABAQBAwABAAADAgAEAAIAAAMABAEAAAEAAQAA"
    "BAMAAwQCAwMAAQMCAwQAAwAAAAEABAAAAAEAAgEAAAAAAAAAAAAAAAAAAgAEAQAAAAEAAAAEAwAA"
    "AQIABAMEAAABAAMAAgEEAAIAAAEEAAABAAABAQAAAAAAAgIAAAAAAAADAgABBAMEAgACBAACBAQA"
    "AgADAAACAgQAAwADAwAEBAQAAAEBAAAABAECAAAAAAAABAACAAAEBAAAAAADAAAEAAMAAAIBAAAA"
    "AAQAAQAABAAAAAACAAEDAwAEBAAAAAAAAAACAQAAAAAEAAIAAAADAAAAAAIAAwAAAAEEBAAAAgAD"
    "AAAAAgEAAAQAAAEAAAAAAAIEAAMAAwQABAACAAEBAAEAAAEABAAAAAICBAQAAQAAAgIEAAAAAAAA"
    "AAAAAAAABAIBAAAAAgIAAAACAQAAAAABAAAAAAQEAgAEAAABAAAAAAAAAAEAAAMCAwAEBAMDBAAA"
    "AAABAAABAAEBAAABAwAAAAABAAABAwMAAAABAAMEAAAAAgAAAAQAAAACAAMAAAAAAAAAAAQAAAQD"
    "AAAABAABAAIAAAIAAAAAAAICAwACAwABAAAAAAQAAwADAgAAAAAAAgEABAIAAAAAAAABBAAAAAIC"
    "AAQAAAQAAAEAAwMDAAAAAQAEBAAAAAEAAAEBAAAAAgAAAwIABAADAAAEAgAAAAAABAAAAAAAAAAC"
    "AAQAAgAEAwAAAAAEAAMEBAEAAQACAAAEAAAABAAAAAAAAAAEAQQAAAQEAAQAAgAAAQEAAQAAAAQE"
    "AAABAAAAAAQABAAEAQAABAACAwACBAQEAAAAAQEAAQABAAAAAAAAAAAAAQAAAQAAAAAEAAACAAAA"
    "BAACAAEAAAAAAAMAAAIAAAMEAQAAAAIBAAIBAAAABAECAAAAAAAAAAABAAMBAAAAAwQAAgAAAwAA"
    "AwAEAQQAAwAAAQQAAwQAAAABAAABAAAEAAQAAAACAAABAAAAAAAAAQIAAAABAAAAAAICAAACAAIA"
    "AAADAgMCAAABAAAAAwACAAMABAAAAAAAAAAAAAAAAAIAAAAAAAQBAAAAAAECAQMBAAAAAAACAAAD"
    "AAAAAAQCAAQBAAACAAAAAAMAAwIAAgMAAAABAwMDBAAABAAEAAAAAAEBAAQCAQAEAAQABAIAAAID"
    "AAEAAQAAAAACAAQAAAABAAADAQECAAAAAAQAAAMABAACAAAAAAQAAAAAAAAAAQEDAAABAwQDAwIA"
    "BAAAAQADAAAAAgAEAwAABAABAQAABAABAAQAAgAAAAAAAAQAAAMBAAACBAAEAAEEAAAABAAABAAA"
    "AAAABAMDAAEBAAAAAAAEAgMAAAAEAgADAAACAgAAAAMAAAQBAQAAAQAEAgAAAAMDAAAAAAABBAAA"
    "AAAAAwQBAAIAAAABAAIAAAIABAMAAAAEAwMAAAABAAAAAwECBAAABAAAAAACAAAAAAAAAAAEAQIB"
    "AAAABAMAAAQCAwEBAgAAAAQAAQAAAAABAAAAAAIAAwACAwECAQAAAgMCAwAEAAAEAQQAAAAAAwAA"
    "AAMAAAMAAAAABAAAAAAAAAMAAAMEAAAAAAAEAAAAAAAAAAQAAwECAAQAAAAAAgAAAAAAAAAAAAAA"
    "AAAEAAADAwAAAAMCAAIAAAAAAwAAAgADAAACAAADAAAAAAMBAAEBAAECAAADAAAEAQMDBAACAAAC"
    "AAABAAACAAQAAAAAAQAAAAAAAQABAwQAAAQCAAAAAwMAAQADAAMAAAMAAAIAAAAAAAAAAAEEAAAA"
    "AAMAAAMEAAACAAAAAAMAAwIAAQMAAgIAAAIAAQAAAAAABAMAAAAAAgEAAAABAQEBAAQAAgQDAAAA"
    "BAMAAAEAAAAAAgIAAwMAAAAABAIAAAADAAECAgIAAAEBAAMBAAQAAgAAAAIAAAIAAAAAAAQEAAAD"
    "AQEEAQIDAAACAAACAAIEAAECAAAAAgMCAwACAAABAwAAAwAAAAAABAAEAAQDAAAAAAABAQEBAAAE"
    "AAAAAwAAAgAAAAADAAECAQMAAAABAAACAAAAAAAAAwMAAAIAAAIAAAEBAAIEAAAEAAAAAAAAAAMA"
    "AQQAAAMEAAMAAwMAAQAAAAAAAAMEAAQCAAIDAAMDBAQAAAAEAAEAAAMCAQACAgAAAAEDAAQAAwAA"
    "AAAAAQQAAAICBAMAAAEAAAAAAAQDAAAAAQAAAQADAAADAAAAAAAAAQAABAAAAAAAAQADAgICAQIA"
    "AAIBAAEAAwAAAAAAAAADAwAAAAAABAIAAAAAAAAEAAMABAAAAAAAAAQAAwQABAAAAAAAAAAAAwED"
    "AAMAAAAAAAAABAMAAAAAAwEAAgABAAAAAQAAAAACAAAAAAAEAQABAAABAQAAAQAAAAMAAgABAAMA"
    "AAAABAAEAQAAAAMABAAAAAEAAQAAAwQDAAACAAQEAAACAAAEBAAAAAMBAAABAAACAAAAAAQAAAAB"
    "AAADAQIBAAADAAEAAQAAAgMBAAADAAIDAAQAAAAAAQEBAQAAAgMAAAACAAAEAwABAAAAAAAEAAAD"
    "AAEEAwEAAQAAAQACAAEAAAMAAQMAAgAAAAIAAAQAAAAAAAIDAAAAAAA="
)


def _rank_to_b64():
    """(debug helper) regenerate _RANK_B64 with jax on CPU."""
    import jax
    import jax.numpy as jnp

    cpu = jax.devices("cpu")[0]
    with jax.default_device(cpu):
        k1, k2 = jax.random.split(jax.random.key(1))
        coin = jax.random.uniform(k1, (B,)) < 0.5
        rank = jnp.where(coin, 0, jax.random.randint(k2, (B,), 0, 5))
    return base64.b64encode(np.asarray(rank, dtype=np.uint8).tobytes()).decode()


_RANK_CACHE = None


def _get_rank() -> np.ndarray:
    """rank[i]: which of the 5 nearest negatives the reference picks per row.

    Must reproduce the reference's jax.random draws bit-exactly. The default
    PRNG impl here is "rbg", whose output is backend-dependent, so compute on
    the CPU backend (the grading reference runs on CPU). Falls back to the
    embedded constant (generated the same way) if jax is unavailable.
    """
    global _RANK_CACHE
    if _RANK_CACHE is not None:
        return _RANK_CACHE
    try:
        import jax
        import jax.numpy as jnp

        cpu = jax.devices("cpu")[0]
        with jax.default_device(cpu):
            k1, k2 = jax.random.split(jax.random.key(1))
            coin = jax.random.uniform(k1, (B,)) < 0.5
            rank = jnp.where(coin, 0, jax.random.randint(k2, (B,), 0, 5))
            r = np.asarray(jax.device_get(rank)).astype(np.uint8)
    except Exception:
        r = np.frombuffer(base64.b64decode(_RANK_B64), dtype=np.uint8)
    assert r.shape == (B,)
    _RANK_CACHE = r
    return r


_NC_CACHE = None


_P2_GRANULES = os.environ.get("P2_GRANULES", "2,5,6,7")


def _p2_set(r):
    """Granules where row-block r uses path P2 (ACT convert).

    Defaults to late granules: early granules run all-DVE while ACT is
    busy with norms; late granules offload to ACT once norms drain."""
    if not _P2_GRANULES:
        return set()
    if _P2_GRANULES == "asym":
        return {2, 6, 7, 4 if r % 2 else 5}
    return {int(t) for t in _P2_GRANULES.split(",")}


def _build_nc():
    import concourse.mybir as mybir
    import concourse.tile as tile
    from concourse import bacc

    F32 = mybir.dt.float32
    BF16 = mybir.dt.bfloat16
    AF = mybir.ActivationFunctionType
    ALU = mybir.AluOpType
    AX = mybir.AxisListType

    nc = bacc.Bacc()
    xa = nc.dram_tensor("xa", [M, D], F32, kind="ExternalInput").ap()
    xp = nc.dram_tensor("xp", [B, D], F32, kind="ExternalInput").ap()
    oh = nc.dram_tensor("oh", [M, 8], BF16, kind="ExternalInput").ap()
    loss = nc.dram_tensor("loss", [128, RB], F32, kind="ExternalOutput").ap()

    with tile.TileContext(nc) as tc:
        with (
            tc.tile_pool(name="const", bufs=1) as constp,
            tc.tile_pool(name="dram", bufs=1, space="DRAM") as dramp,
            tc.tile_pool(name="stage", bufs=8) as stagep,
            tc.tile_pool(name="scaled", bufs=6) as scaledp,
            tc.tile_pool(name="ptpool", bufs=3) as ptpool,
            tc.tile_pool(name="small", bufs=4) as smallp,
            tc.tile_pool(name="conv", bufs=6) as convp,
            tc.tile_pool(name="fold", bufs=4) as foldp,
            tc.tile_pool(name="cand", bufs=1) as candp,
            tc.tile_pool(name="psg", bufs=3, space="PSUM") as psg,
            tc.tile_pool(name="pstr", bufs=2, space="PSUM") as pstr,
        ):
            # ---------------- constants ----------------
            # ACT table warmup: touch every activation function used later so
            # the table loads happen during the DMA preamble, not in the
            # chunk-0 critical chain
            warm = constp.tile([128, 1], F32)
            nc.vector.memset(warm, 1.0)
            warm2 = constp.tile([128, 1], F32)
            nc.scalar.activation(warm2, warm, AF.Square)
            nc.scalar.activation(warm2, warm, AF.Sqrt)
            nc.scalar.activation(warm2, warm, AF.Relu)
            nc.scalar.copy(warm2, warm)

            negid_f = constp.tile([128, 128], F32)
            nc.gpsimd.memset(negid_f, 0.0)
            nc.gpsimd.affine_select(
                out=negid_f,
                in_=negid_f,
                compare_op=ALU.not_equal,
                fill=NEG_BIG,
                base=0,
                pattern=[[-1, 128]],
                channel_multiplier=1,
            )
            negid = constp.tile([128, 128], BF16)
            nc.scalar.copy(negid, negid_f)
            ident_f = constp.tile([128, 128], F32)
            nc.gpsimd.memset(ident_f, 0.0)
            nc.gpsimd.affine_select(
                out=ident_f,
                in_=ident_f,
                compare_op=ALU.not_equal,
                fill=1.0,
                base=0,
                pattern=[[-1, 128]],
                channel_multiplier=1,
            )
            identb = constp.tile([128, 128], BF16)
            nc.scalar.copy(identb, ident_f)

            oh_sb = constp.tile([128, RB * 8], BF16)

            def emit_load_oh():
                nc.sync.dma_start(
                    oh_sb.rearrange("p (r k) -> p r k", r=RB),
                    oh.rearrange("(r p) k -> p r k", p=128),
                )

            # ---------------- anchor slab prep (emitted via plan) -----------
            xa_res = constp.tile([128, RB * D], F32)
            sq_scr = smallp.tile([128, D], F32, tag="sqscr")
            na2 = constp.tile([128, RB], F32)
            na_half = constp.tile([128, RB], F32)
            inv2na = constp.tile([128, RB], F32)
            xa_s = constp.tile([128, RB * D], BF16)
            aT = [constp.tile([128, M], BF16, name=f"aT{k}") for k in range(2)]

            def emit_load_xa():
                nc.sync.dma_start(
                    xa_res.rearrange("p (r d) -> p r d", r=RB),
                    xa.rearrange("(r p) d -> p r d", p=128),
                )

            def emit_proc_xa():
                # norms on ACT; scale + writeback ride the (idle) DVE queue
                # so the Pool queue stays free for the chunk pipeline
                for r in range(RB):
                    nc.scalar.activation(
                        sq_scr,
                        xa_res[:, r * D : (r + 1) * D],
                        AF.Square,
                        accum_out=na2[:, r : r + 1],
                    )
                nc.scalar.activation(na_half, na2, AF.Sqrt, scale=0.25)
                nc.vector.reciprocal(inv2na, na_half)
                for r in range(RB):
                    nc.vector.tensor_scalar_mul(
                        xa_s[:, r * D : (r + 1) * D],
                        xa_res[:, r * D : (r + 1) * D],
                        inv2na[:, r : r + 1],
                    )


            # ---------------- local-positive head (for s_ii row dots) -------
            xp_head = constp.tile([128, RB * D], F32)
            nc.sync.dma_start(
                xp_head.rearrange("p (r d) -> p r d", r=RB),
                xp[:M].rearrange("(r p) d -> p r d", p=128),
            )
            # ---------------- positives: chunked norm+scale+writeback -------
            rawii = constp.tile([128, RB], F32)
            np2 = constp.tile([128, CH * 4], F32)
            nps = constp.tile([128, CH * 4], F32)
            invnp = constp.tile([128, CH * 4], F32)
            # per-granule internal DRAM staging for exact wb->transpose deps
            xps_d = [
                dramp.tile([GW, D], BF16, name=f"xpsd{g}") for g in range(NG)
            ]
            # The SP DMA queue is in-order with head-of-line blocking, so
            # its issue order is planned explicitly: chunk loads stream
            # early (no waits), the aT transposes slot in once xa's
            # writeback lands, and each granule's pT transposes follow the
            # writebacks they consume. This keeps granule g's operands
            # ready ahead of the main loop's consumption rate.
            stages = {}

            def emit_load(c):
                stage = stagep.tile([128, CW // 128 * D], F32, tag="xpstage")
                nc.sync.dma_start(
                    stage.rearrange("p (s d) -> p s d", s=CW // 128),
                    xp[c * CW : (c + 1) * CW].rearrange("(s p) d -> p s d", p=128),
                )
                stages[c] = stage

            def emit_aT():
                # PE transposes + DVE evicts: PE/DVE are idle during fill,
                # and this skips the DRAM round-trip entirely
                for k in range(2):
                    ptile = pstr.tile([128, GW], BF16, tag="trps", name=f"aTps{k}")
                    for r in range(RB):
                        nc.tensor.transpose(
                            ptile[:, r * 128 : (r + 1) * 128],
                            xa_s[:, r * D + k * 128 : r * D + k * 128 + 128],
                            identb,
                        )
                    nc.vector.tensor_copy(aT[k], ptile)

            def emit_tr(g):
                ptk = [
                    ptpool.tile([128, GW], BF16, tag=f"pt{k}", name=f"pT{g}_{k}")
                    for k in range(2)
                ]
                if g < 2:
                    for k in range(2):
                        ptile = pstr.tile(
                            [128, GW], BF16, tag="trps", name=f"pTps{g}_{k}"
                        )
                        for c_half in range(2):
                            xps = xps_t[2 * g + c_half]
                            for s in range(CW // 128):
                                j = c_half * 4 + s
                                nc.tensor.transpose(
                                    ptile[:, j * 128 : (j + 1) * 128],
                                    xps[:, s * D + k * 128 : s * D + k * 128 + 128],
                                    identb,
                                )
                        nc.vector.tensor_copy(ptk[k], ptile)
                else:
                    for k in range(2):
                        nc.sync.dma_start(
                            ptk[k],
                            xps_d[g][:, k * 128 : (k + 1) * 128],
                            transpose=True,
                        )
                pT[g] = ptk

            pT = {}

            def emit_rawii():
                # s_ii row dots from the raw local-positive rows (chunks 0/1).
                # Pool muls run during fill (Pool is idle then); placement
                # also releases the c0/c1 stage slots early so later loads
                # never block on them.
                dots = []
                for c in range(2):
                    for s in range(CW // 128):
                        r = c * 4 + s
                        dot_scr = smallp.tile([128, D], F32, tag=f"dotscr{r}")
                        nc.gpsimd.tensor_mul(
                            dot_scr,
                            xa_res[:, r * D : (r + 1) * D],
                            stages[c][:, s * D : (s + 1) * D],
                        )
                        dots.append((r, dot_scr))
                for r, dot_scr in dots:
                    nc.vector.reduce_sum(rawii[:, r : r + 1], dot_scr, axis=AX.X)

            xps_t = {}

            def emit_proc(c):
                # norms on ACT; scale on DVE for the critical first chunks
                # (DVE is idle during fill), Pool for the steady-state rest
                stage = stages[c]
                sq2 = smallp.tile([128, D], F32, tag="sqscr2")
                for s in range(CW // 128):
                    b = c * 4 + s
                    nc.scalar.activation(
                        sq2,
                        stage[:, s * D : (s + 1) * D],
                        AF.Square,
                        accum_out=np2[:, b : b + 1],
                    )
                nc.scalar.activation(
                    nps[:, c * 4 : (c + 1) * 4],
                    np2[:, c * 4 : (c + 1) * 4],
                    AF.Sqrt,
                )
                nc.vector.reciprocal(
                    invnp[:, c * 4 : (c + 1) * 4], nps[:, c * 4 : (c + 1) * 4]
                )
                xps = scaledp.tile([128, CW // 128 * D], BF16, tag="xps")
                xps_t[c] = xps
                eng = nc.vector if c < 2 else nc.gpsimd
                for s in range(CW // 128):
                    b = c * 4 + s
                    eng.tensor_scalar_mul(
                        xps[:, s * D : (s + 1) * D],
                        stage[:, s * D : (s + 1) * D],
                        invnp[:, b : b + 1],
                    )
                if c >= 4:
                    emit_wb(c, nc.gpsimd)

            def emit_wb(c, eng):
                g, half = c // 2, c % 2
                eng.dma_start(
                    xps_d[g][half * CW : (half + 1) * CW].rearrange(
                        "(s p) d -> p s d", p=128
                    ),
                    xps_t[c].rearrange("p (s d) -> p s d", s=CW // 128),
                )

            # plan: c0/xa/c1 chains first (granule 0 + aT are the critical
            # path; their scales ride the idle DVE queue and their
            # writebacks the SP queue), rawii early on the idle Pool,
            # then the steady-state chunk pipeline on ACT/Pool
            plan = ["L0", "L1", "LXA", "L2", "L3", "P0", "PXA", "AT",
                    "P1", "T0", "L4", "L5", "PR", "P2", "P3", "T1",
                    "L6", "L7", "P4", "P5", "T2",
                    "L8", "L9", "P6", "P7", "T3", "L10", "L11", "P8", "P9",
                    "T4", "L12", "L13", "P10", "P11", "T5", "L14", "L15",
                    "P12", "P13", "T6", "OH", "P14", "P15", "T7"]
            for step in plan:
                if step == "AT":
                    emit_aT()
                elif step == "LXA":
                    emit_load_xa()
                elif step == "PXA":
                    emit_proc_xa()
                elif step == "PR":
                    emit_rawii()
                elif step == "OH":
                    emit_load_oh()
                elif step == "WBXA":
                    emit_wb_xa()
                elif step.startswith("WB"):
                    emit_wb(int(step[2:]), nc.sync)
                elif step.startswith("L"):
                    emit_load(int(step[1:]))
                elif step.startswith("P"):
                    emit_proc(int(step[1:]))
                else:
                    emit_tr(int(step[1:]))

            # ---------------- per-row epilogue (interleaved with g=7) -------
            loss_sb = constp.tile([128, RB], F32)
            sii = constp.tile([128, RB], F32)

            def emit_row_epilogue(r):
                # s_ii = rawii * (2/na) * (1/np)  (local rows = first RB blocks)
                nc.vector.tensor_scalar(
                    sii[:, r : r + 1],
                    rawii[:, r : r + 1],
                    inv2na[:, r : r + 1],
                    invnp[:, r : r + 1],
                    op0=ALU.mult,
                    op1=ALU.mult,
                )
                top8 = smallp.tile([128, 8], BF16, tag="top8")
                nc.vector.max(out=top8, in_=cand[r][:, slot_off[r][NG - 1] : CANDW])
                sel_scr = smallp.tile([128, 8], BF16, tag="selscr")
                nc.vector.tensor_mul(sel_scr, top8, oh_sb[:, r * 8 : (r + 1) * 8])
                selv = smallp.tile([128, 1], F32, tag="selv")
                nc.vector.reduce_sum(selv, sel_scr, axis=AX.X)
                nc.vector.tensor_sub(loss_sb[:, r : r + 1], selv, sii[:, r : r + 1])

            # ---------------- main loop: matmul granules + scan -------------
            # every unit reduces its [128,1024] slab to a 128-wide bf16
            # group-max remnant (groups of 8 columns); cand[r] = 8 x 128
            # tightly packed candidate slots: A units (direct Max8) yield 8
            # values, P2 units (convert+fold) yield 128 group-maxes. The
            # premerged top8 of granules 0..6 lands after all slots, so the
            # final merge only reads [off(7) : width].
            slot_off = []  # per (r, g)
            CANDW = PM_OFF = None
            for r in range(RB):
                offs, off = [], 0
                for g in range(NG):
                    offs.append(off)
                    off += 128 if g in _p2_set(r) else 8
                slot_off.append(offs)
                assert PM_OFF is None or PM_OFF == off
                PM_OFF = off
                CANDW = off + 8  # + premerge slot
            cand = [
                candp.tile([128, CANDW], BF16, tag=f"cand{r}", name=f"cand{r}")
                for r in range(RB)
            ]
            for g in range(NG):
                for r in range(RB):
                    is_p2 = g in _p2_set(r)
                    gt = psg.tile([128, GW], F32)
                    dh = r // 4 if g == 0 else -1  # half holding the diagonal
                    for h in range(2):
                        for k in range(2):
                            nc.tensor.matmul(
                                gt[:, h * 512 : (h + 1) * 512],
                                aT[k][:, r * 128 : (r + 1) * 128],
                                pT[g][k][:, h * 512 : (h + 1) * 512],
                                start=(k == 0),
                                stop=(k == 1 and h != dh),
                            )
                        if h == dh:
                            # rotated layout: row block r's self-cols are
                            # [r*128, r*128+128) of granule 0 on every core.
                            # Accumulate -3e38*I there via the PE itself.
                            nc.tensor.matmul(
                                gt[:, r * 128 : r * 128 + 128],
                                negid,
                                identb,
                                start=False,
                                stop=True,
                            )
                    so = slot_off[r][g]
                    if is_p2:
                        # P2: ACT converts the slab to bf16 SBUF; DVE folds
                        # it 8:1 (a PSUM operand pair is illegal on DVE, so
                        # the convert hop is required for tensor_max)
                        conv = convp.tile([128, GW], BF16, tag="conv")
                        nc.scalar.copy(conv, gt)
                        fold1 = foldp.tile([128, 512], BF16, tag="fold1")
                        nc.vector.tensor_max(fold1, conv[:, 0:512], conv[:, 512:1024])
                        fold2 = foldp.tile([128, 256], BF16, tag="fold2")
                        nc.vector.tensor_max(fold2, fold1[:, 0:256], fold1[:, 256:512])
                        nc.vector.tensor_max(
                            cand[r][:, so : so + 128],
                            fold2[:, 0:128],
                            fold2[:, 128:256],
                        )
                    else:
                        # A: Max8 straight from PSUM (single-input op)
                        nc.vector.max(out=cand[r][:, so : so + 8], in_=gt)
                    if g == NG - 2:
                        nc.vector.max(
                            out=cand[r][:, PM_OFF : PM_OFF + 8],
                            in_=cand[r][:, 0 : slot_off[r][NG - 1]],
                        )
                    elif g == NG - 1:
                        emit_row_epilogue(r)

            relu_sb = constp.tile([128, RB], F32)
            nc.scalar.activation(relu_sb, loss_sb, AF.Relu)
            nc.sync.dma_start(loss, relu_sb)

    nc.compile()
    return nc


def _get_nc():
    global _NC_CACHE
    if _NC_CACHE is None:
        _NC_CACHE = _build_nc()
    return _NC_CACHE


def kernel(x: np.ndarray, _want_timing: bool = False):
    """x: [8192, 2, 256] float32 -> scalar float32 loss (0-d ndarray)."""
    import ml_dtypes
    from concourse.bass_utils import run_bass_kernel_spmd

    x = np.ascontiguousarray(np.asarray(x, dtype=np.float32))
    assert x.shape == (B, 2, D)
    x0 = x[:, 0, :]
    x1 = np.ascontiguousarray(x[:, 1, :])

    rank = _get_rank()
    onehot = np.zeros((B, 8), dtype=ml_dtypes.bfloat16)
    onehot[np.arange(B), rank] = 1.0

    in_maps = []
    for c in range(NCORES):
        lo = c * M
        in_maps.append(
            {
                "xa": np.ascontiguousarray(x0[lo : lo + M]),
                "xp": np.ascontiguousarray(np.roll(x1, -lo, axis=0)),
                "oh": np.ascontiguousarray(onehot[lo : lo + M]),
            }
        )

    nc = _get_nc()
    res = run_bass_kernel_spmd(nc, in_maps, list(range(NCORES)))
    per_row = np.concatenate(
        [res.results[c]["loss"].T.reshape(M) for c in range(NCORES)]
    )  # loss[p, r] -> row r*128+p; .T gives [r, p] -> flat local rows
    out = np.float32(np.mean(per_row))
    if _want_timing:
        return np.asarray(out), res, per_row
    return np.asarray(out)


if __name__ == "__main__":
    rng = np.random.default_rng(0)
    x = rng.standard_normal((B, 2, D)).astype(np.float32)
    print(kernel(x))
